# revision 13
# baseline (speedup 1.0000x reference)
"""RWKV GPT block kernel for 8 Trainium2 NeuronCores.

Sharding: data-parallel over (batch, seq-half) = 8 shards, each with a
2-token halo at the front (zero-padded at sequence start). No collectives.

Key simplification: the reference's `_wkv_run` with aa=bb=0, pp=-1e38 reduces
to wkv == v exactly (e1 = exp(-1e38) = 0, e2 = exp(0) = 1), so Wk/tm_first/
tm_decay never affect the output and the k-projection is skipped.

On-chip layout is channel-major [H partitions, tokens free]: the time shift
is a column offset, per-channel mix coefficients are per-partition scalars,
and activations feed matmuls directly as the moving operand. LayerNorm
statistics are computed with ones-vector matmuls on the PE and broadcast
back across partitions with K=1 matmuls.
"""
import sys

sys.path.insert(0, "/opt/trn_rl_repo")
sys.path.insert(0, "/opt/pypackages")

import numpy as np

H = 2048
KT = H // 128          # 16 contraction tiles
OT = H // 128          # 16 output tiles
UPT = 4 * H // 128     # 64 FFN up tiles
B = 4
T = 2048
TCORE = 1026           # 2 halo + 1024 tokens per core
EPS = 1e-5
INV_H = 1.0 / H

# chunk col ranges over the 1026 token columns (h/apply tiles cover
# [max(a-1,lo), b)); part-1 valid cols start at 0, part-2 (x1) at 1.
CH1 = [(1, 342), (342, 684), (684, 1026)]   # vin/rin/oin matmul col ranges
ST1 = [(0, 342), (342, 684), (684, 1026)]   # LN1 stats col ranges
ST2 = [(1, 342), (342, 684), (684, 1026)]   # LN2 stats col ranges
P2 = [(0, 512), (512, 1024)]                # part-2 cols (token col - 2)

_BUILD_CACHE = {}


def _np16(a):
    return np.ascontiguousarray(np.asarray(a, dtype=np.float32).astype(np.float16))


def _prep_w(WT, dt=np.float16):
    """[in, out] weight -> panel layout [out_tiles, 128, in] such that
    panel[oi][p, k*128+n] = WT[k*128+p, oi*128+n] (lhsT tiles slice out)."""
    IN, OUT = WT.shape
    kt, ot = IN // 128, OUT // 128
    a = WT.reshape(kt, 128, ot, 128).transpose(2, 1, 0, 3).reshape(ot, 128, IN)
    return np.ascontiguousarray(a.astype(dt))


def _mix128(v):
    """[H] per-channel vector -> [128, KT] (partition-major)."""
    return np.ascontiguousarray(
        np.asarray(v, dtype=np.float32).reshape(-1)[:H].reshape(KT, 128).T
    )


def build():
    import contextlib

    import concourse.bacc as bacc
    import concourse.mybir as mybir
    import concourse.tile as tile

    F16 = mybir.dt.float16
    F32 = mybir.dt.float32
    AF = mybir.ActivationFunctionType
    OP = mybir.AluOpType

    nc = bacc.Bacc("TRN2", target_bir_lowering=False)

    xT = nc.dram_tensor("xT", [H, TCORE], F16, kind="ExternalInput")
    Wv = nc.dram_tensor("Wv", [OT, 128, H], F16, kind="ExternalInput")
    Wr = nc.dram_tensor("Wr", [OT, 128, H], F16, kind="ExternalInput")
    Wo = nc.dram_tensor("Wo", [OT, 128, H], F16, kind="ExternalInput")
    Wkey = nc.dram_tensor("Wkey", [UPT, 128, H], F16, kind="ExternalInput")
    Wval = nc.dram_tensor("Wval", [OT, 128, 4 * H], F16, kind="ExternalInput")
    Wcr = nc.dram_tensor("Wcr", [OT, 128, H], F16, kind="ExternalInput")
    mixv = nc.dram_tensor("mixv", [128, KT], F32, kind="ExternalInput")
    mixr = nc.dram_tensor("mixr", [128, KT], F32, kind="ExternalInput")
    mixk = nc.dram_tensor("mixk", [128, KT], F32, kind="ExternalInput")
    out = nc.dram_tensor("out", [H, 1024], F32, kind="ExternalOutput")
    x1d = nc.dram_tensor("x1d", [H, TCORE], F32, kind="Internal")
    x1h = nc.dram_tensor("x1h", [H, TCORE], F16, kind="Internal")

    with tile.TileContext(nc) as tc, contextlib.ExitStack() as g:
        cpool = g.enter_context(tc.tile_pool(name="consts", bufs=1))
        st = g.enter_context(tc.tile_pool(name="st", bufs=1, space="PSUM"))
        bc = g.enter_context(tc.tile_pool(name="bc", bufs=1, space="PSUM"))
        mm = g.enter_context(tc.tile_pool(name="mm", bufs=4, space="PSUM"))
        rows = g.enter_context(tc.tile_pool(name="rows", bufs=3))
        rsc = g.enter_context(tc.tile_pool(name="rsc", bufs=1))
        bcs = g.enter_context(tc.tile_pool(name="bcs", bufs=2))
        xck = g.enter_context(tc.tile_pool(name="xck", bufs=20))
        sqp = g.enter_context(tc.tile_pool(name="sqp", bufs=3))

        ones_c = cpool.tile([128, 1], F16)
        nc.vector.memset(ones_c[:], 1.0)
        ones_r = cpool.tile([1, 128], F32)
        nc.vector.memset(ones_r[:], 1.0)
        mv = cpool.tile([128, KT], F32)
        nc.sync.dma_start(mv[:], mixv[:])
        mr = cpool.tile([128, KT], F32)
        nc.sync.dma_start(mr[:], mixr[:])
        mk = cpool.tile([128, KT], F32)
        nc.sync.dma_start(mk[:], mixk[:])

        def ln_chunk(src, lo, stats_rng, prev, apply_fn):
            """One LayerNorm chunk: DMA the 16 k-tiles covering
            [max(sa-1,lo), sb), ones-matmul stats over [sa, sb), row math into
            per-chunk [1,n] row tiles, K=1 broadcast over [ha, sb) (first
            column taken from the previous chunk's rows), then
            apply_fn(ki, xt, hn, ha, ab, cb) per tile. Returns (a_rc, c_rc)."""
            sa, sb = stats_rng
            ha = max(sa - 1, lo)
            hn = sb - ha
            off = sa - ha
            n = sb - sa
            tiles = []
            s1 = st.tile([1, 512], F32, tag="s1")
            s2 = st.tile([1, 512], F32, tag="s2")
            for ki in range(KT):
                xt = xck.tile([128, 512], F16, tag="xck")
                nc.sync.dma_start(xt[:, :hn], src[ki * 128 : (ki + 1) * 128, ha:sb])
                tiles.append(xt)
                xs = xt[:, off : off + n]
                nc.tensor.matmul(s1[:, :n], ones_c[:], xs,
                                 start=(ki == 0), stop=(ki == KT - 1))
                sq = sqp.tile([128, 512], F16, tag="sq")
                nc.scalar.square(sq[:, :n], xs)
                nc.tensor.matmul(s2[:, :n], ones_c[:], sq[:, :n],
                                 start=(ki == 0), stop=(ki == KT - 1))
            # row math: a = 1/sqrt(E[x^2]-E[x]^2+eps), c = -mean*a
            m = rsc.tile([1, 512], F32, tag="m")
            nc.vector.tensor_scalar_mul(m[:, :n], s1[:, :n], INV_H)
            var = rsc.tile([1, 512], F32, tag="var")
            nc.vector.tensor_scalar_mul(var[:, :n], s2[:, :n], INV_H)
            msq = rsc.tile([1, 512], F32, tag="msq")
            nc.vector.tensor_mul(msq[:, :n], m[:, :n], m[:, :n])
            nc.vector.tensor_sub(var[:, :n], var[:, :n], msq[:, :n])
            nc.vector.tensor_scalar_add(var[:, :n], var[:, :n], EPS)
            sd = rsc.tile([1, 512], F32, tag="sd")
            nc.scalar.sqrt(sd[:, :n], var[:, :n])
            a_rc = rows.tile([1, 512], F32, tag="arow")
            c_rc = rows.tile([1, 512], F32, tag="crow")
            nc.vector.reciprocal(a_rc[:, :n], sd[:, :n])
            nc.vector.scalar_tensor_tensor(
                c_rc[:, :n], m[:, :n], -1.0, a_rc[:, :n],
                op0=OP.mult, op1=OP.mult)
            # broadcast a/c over [ha, sb) across partitions; when the range
            # starts one column before this chunk, that column's value comes
            # from the previous chunk's row tiles
            abp = bc.tile([128, 512], F32, tag="abp")
            cbp = bc.tile([128, 512], F32, tag="cbp")
            if off:
                pa_rc, pc_rc, pn = prev
                nc.tensor.matmul(abp[:, 0:1], ones_r[:], pa_rc[:, pn - 1 : pn],
                                 start=True, stop=True, skip_group_check=True)
                nc.tensor.matmul(cbp[:, 0:1], ones_r[:], pc_rc[:, pn - 1 : pn],
                                 start=True, stop=True, skip_group_check=True)
            nc.tensor.matmul(abp[:, off : off + n], ones_r[:], a_rc[:, :n],
                             start=True, stop=True, skip_group_check=True)
            nc.tensor.matmul(cbp[:, off : off + n], ones_r[:], c_rc[:, :n],
                             start=True, stop=True, skip_group_check=True)
            ab = bcs.tile([128, 512], F32, tag="ab")
            nc.scalar.copy(ab[:, :hn], abp[:, :hn])
            cb = bcs.tile([128, 512], F32, tag="cb")
            nc.scalar.copy(cb[:, :hn], cbp[:, :hn])
            for ki in range(KT):
                apply_fn(ki, tiles[ki], hn, ha, ab, cb)
            return a_rc, c_rc, n

        # ================= part 1: time-mix =================
        with tc.tile_pool(name="oinp", bufs=1) as oinp:
            oin = oinp.tile([128, KT, TCORE], F16)
            with tc.tile_pool(name="vinp", bufs=1) as vinp, \
                 tc.tile_pool(name="rinp", bufs=1) as rinp, \
                 tc.tile_pool(name="p1sc", bufs=3) as p1sc, \
                 tc.tile_pool(name="hp", bufs=3) as hp, \
                 tc.tile_pool(name="wvp", bufs=2) as wvp, \
                 tc.tile_pool(name="wrp", bufs=2) as wrp, \
                 tc.tile_pool(name="sgp", bufs=2) as sgp:
                vin = vinp.tile([128, KT, TCORE], F16)
                rin = rinp.tile([128, KT, TCORE], F16)

                def apply1(ki, xt, hn, ha, ab, cb):
                    tt = p1sc.tile([128, 512], F32, tag="tt")
                    nc.vector.tensor_mul(tt[:, :hn], xt[:, :hn], ab[:, :hn])
                    h = hp.tile([128, 512], F16, tag="h")
                    nc.vector.tensor_add(h[:, :hn], tt[:, :hn], cb[:, :hn])
                    nmix = hn - 1
                    d = p1sc.tile([128, 512], F16, tag="d")
                    nc.vector.tensor_sub(d[:, :nmix], h[:, 1:hn], h[:, :nmix])
                    nc.vector.scalar_tensor_tensor(
                        vin[:, ki, ha + 1 : ha + hn], d[:, :nmix],
                        mv[:, ki : ki + 1], h[:, :nmix], op0=OP.mult, op1=OP.add)
                    nc.vector.scalar_tensor_tensor(
                        rin[:, ki, ha + 1 : ha + hn], d[:, :nmix],
                        mr[:, ki : ki + 1], h[:, :nmix], op0=OP.mult, op1=OP.add)

                prev = None
                for rng in ST1:
                    prev = ln_chunk(xT, 0, rng, prev, apply1)

                # v/r matmuls + sigmoid(r) * v -> oin
                for oi in range(OT):
                    wv_t = wvp.tile([128, H], F16, tag="wv")
                    nc.sync.dma_start(wv_t[:], Wv[oi])
                    wr_t = wrp.tile([128, H], F16, tag="wr")
                    nc.sync.dma_start(wr_t[:], Wr[oi])
                    for pa, pb in CH1:
                        n = pb - pa
                        vps = mm.tile([128, 512], F32, tag="acc")
                        for ki in range(KT):
                            nc.tensor.matmul(
                                vps[:, :n], wv_t[:, ki * 128 : (ki + 1) * 128],
                                vin[:, ki, pa:pb],
                                start=(ki == 0), stop=(ki == KT - 1))
                        rps = mm.tile([128, 512], F32, tag="acc")
                        for ki in range(KT):
                            nc.tensor.matmul(
                                rps[:, :n], wr_t[:, ki * 128 : (ki + 1) * 128],
                                rin[:, ki, pa:pb],
                                start=(ki == 0), stop=(ki == KT - 1))
                        sg = sgp.tile([128, 512], F16, tag="sg")
                        nc.scalar.activation(sg[:, :n], rps[:, :n], AF.Sigmoid)
                        nc.vector.tensor_mul(oin[:, oi, pa:pb], sg[:, :n],
                                             vps[:, :n])

            # o-projection + residual -> x1 (fp32 + fp16 copies to DRAM)
            with tc.tile_pool(name="wop", bufs=2) as wop, \
                 tc.tile_pool(name="csc", bufs=4) as csc:
                for oi in range(OT):
                    wo_t = wop.tile([128, H], F16, tag="wo")
                    nc.sync.dma_start(wo_t[:], Wo[oi])
                    for pa, pb in CH1:
                        n = pb - pa
                        ops_ = mm.tile([128, 512], F32, tag="acc")
                        for ki in range(KT):
                            nc.tensor.matmul(
                                ops_[:, :n], wo_t[:, ki * 128 : (ki + 1) * 128],
                                oin[:, ki, pa:pb],
                                start=(ki == 0), stop=(ki == KT - 1))
                        xt = csc.tile([128, 512], F16, tag="xs3")
                        nc.sync.dma_start(
                            xt[:, :n], xT[oi * 128 : (oi + 1) * 128, pa:pb])
                        x1t = csc.tile([128, 512], F32, tag="x1t")
                        nc.vector.tensor_add(x1t[:, :n], ops_[:, :n], xt[:, :n])
                        nc.sync.dma_start(
                            x1d[oi * 128 : (oi + 1) * 128, pa:pb], x1t[:, :n])
                        x1t16 = csc.tile([128, 512], F16, tag="x1t16")
                        nc.vector.tensor_copy(x1t16[:, :n], x1t[:, :n])
                        nc.sync.dma_start(
                            x1h[oi * 128 : (oi + 1) * 128, pa:pb], x1t16[:, :n])

        # ================= part 2: channel-mix =================
        with tc.tile_pool(name="cmp", bufs=1) as cmp_:
            cm = cmp_.tile([128, KT, 1024], F16)
            with tc.tile_pool(name="dsc", bufs=3) as dsc, \
                 tc.tile_pool(name="h2p", bufs=3) as h2p:
                def apply2(ki, xt, hn, ha, ab, cb):
                    tt = dsc.tile([128, 512], F32, tag="tt2")
                    nc.vector.tensor_mul(tt[:, :hn], xt[:, :hn], ab[:, :hn])
                    h2 = h2p.tile([128, 512], F16, tag="h2")
                    nc.vector.tensor_add(h2[:, :hn], tt[:, :hn], cb[:, :hn])
                    nmix = hn - 1
                    d2 = dsc.tile([128, 512], F16, tag="d2")
                    nc.vector.tensor_sub(d2[:, :nmix], h2[:, 1:hn], h2[:, :nmix])
                    nc.vector.scalar_tensor_tensor(
                        cm[:, ki, ha - 1 : ha - 1 + nmix], d2[:, :nmix],
                        mk[:, ki : ki + 1], h2[:, :nmix], op0=OP.mult, op1=OP.add)

                prev = None
                for rng in ST2:
                    prev = ln_chunk(x1h, 1, rng, prev, apply2)

            # FFN in 2 token-halves (Wkey/Wval/Wcr streamed per half)
            with tc.tile_pool(name="silup", bufs=1) as silup, \
                 tc.tile_pool(name="wkp", bufs=2) as wkp, \
                 tc.tile_pool(name="wvalp", bufs=3) as wvalp, \
                 tc.tile_pool(name="wcp", bufs=2) as wcp, \
                 tc.tile_pool(name="fsc", bufs=2) as fsc:
                sil = silup.tile([128, UPT, 512], F16)
                for pa, pb in P2:
                    for ui in range(UPT):
                        wk_t = wkp.tile([128, H], F16, tag="wk")
                        nc.sync.dma_start(wk_t[:], Wkey[ui])
                        kps = mm.tile([128, 512], F32, tag="acc")
                        for ki in range(KT):
                            nc.tensor.matmul(
                                kps[:], wk_t[:, ki * 128 : (ki + 1) * 128],
                                cm[:, ki, pa:pb],
                                start=(ki == 0), stop=(ki == KT - 1))
                        sgk = fsc.tile([128, 512], F16, tag="sgk")
                        nc.scalar.activation(sgk[:], kps[:], AF.Sigmoid)
                        nc.vector.tensor_mul(sil[:, ui, :], sgk[:], kps[:])
                    for oi in range(OT):
                        wva0 = wvalp.tile([128, 2 * H], F16, tag="wva")
                        nc.sync.dma_start(wva0[:], Wval[oi, :, : 2 * H])
                        wva1 = wvalp.tile([128, 2 * H], F16, tag="wva")
                        nc.sync.dma_start(wva1[:], Wval[oi, :, 2 * H :])
                        kvps = mm.tile([128, 512], F32, tag="acc")
                        for ki in range(UPT):
                            wva = wva0 if ki < UPT // 2 else wva1
                            kj = ki % (UPT // 2)
                            nc.tensor.matmul(
                                kvps[:], wva[:, kj * 128 : (kj + 1) * 128],
                                sil[:, ki, :],
                                start=(ki == 0), stop=(ki == UPT - 1))
                        wc_t = wcp.tile([128, H], F16, tag="wc")
                        nc.sync.dma_start(wc_t[:], Wcr[oi])
                        rrps = mm.tile([128, 512], F32, tag="acc")
                        for ki in range(KT):
                            nc.tensor.matmul(
                                rrps[:], wc_t[:, ki * 128 : (ki + 1) * 128],
                                cm[:, ki, pa:pb],
                                start=(ki == 0), stop=(ki == KT - 1))
                        sr = fsc.tile([128, 512], F16, tag="sr")
                        nc.scalar.activation(sr[:], rrps[:], AF.Sigmoid)
                        prod = fsc.tile([128, 512], F32, tag="prod")
                        nc.vector.tensor_mul(prod[:], sr[:], kvps[:])
                        x1t = fsc.tile([128, 512], F32, tag="x1r")
                        nc.sync.dma_start(
                            x1t[:],
                            x1d[oi * 128 : (oi + 1) * 128, pa + 2 : pb + 2])
                        nc.vector.tensor_add(prod[:], prod[:], x1t[:])
                        nc.sync.dma_start(
                            out[oi * 128 : (oi + 1) * 128, pa:pb], prod[:])
    nc.compile()
    return nc


def get_nc():
    if "nc" not in _BUILD_CACHE:
        _BUILD_CACHE["nc"] = build()
    return _BUILD_CACHE["nc"]


def make_in_maps(inputs):
    x = np.asarray(inputs["x"], dtype=np.float32)
    shared = {
        "Wv": _prep_w(np.asarray(inputs["Wv"], np.float32).T),
        "Wr": _prep_w(np.asarray(inputs["Wr"], np.float32).T),
        "Wo": _prep_w(np.asarray(inputs["Wo"], np.float32).T),
        "Wkey": _prep_w(np.asarray(inputs["Wkey"], np.float32).T),
        "Wval": _prep_w(np.asarray(inputs["Wval"], np.float32).T),
        "Wcr": _prep_w(np.asarray(inputs["Wcr"], np.float32).T),
        "mixv": _mix128(inputs["tm_mv"]),
        "mixr": _mix128(inputs["tm_mr"]),
        "mixk": _mix128(inputs["cm_mk"]),
    }
    in_maps = []
    for c in range(8):
        b, half = divmod(c, 2)
        s = half * 1024
        xs = np.zeros((TCORE, H), np.float32)
        lo = max(s - 2, 0)
        xs[2 - (s - lo) :, :] = x[b, lo : s + 1024, :]
        m = dict(shared)
        m["xT"] = _np16(xs.T)
        in_maps.append(m)
    return in_maps


def run(inputs, **kw):
    from concourse.bass_utils import run_bass_kernel_spmd

    in_maps = make_in_maps(inputs)
    nc = get_nc()
    res = run_bass_kernel_spmd(nc, in_maps, core_ids=list(range(8)), **kw)
    outa = np.empty((B, T, H), np.float32)
    for c in range(8):
        b, half = divmod(c, 2)
        outa[b, half * 1024 : (half + 1) * 1024, :] = res.results[c]["out"].T
    return outa, res


def kernel(**inputs):
    return run(inputs)[0]
